# revision 13
# baseline (speedup 1.0000x reference)
"""Trainium2 Bass kernel for nn_DistanceLayer (gaussian-prior distance attention).

Math: out[b,i] = sum_j softmax_j(q_i.k_j * MD^-0.5 * prior(j-i))[j] * (j-i)

Key observation: the gaussian prior (std=1) underflows so fast in f32 that
for |j-i| outside a small band the f32 score is exactly 0, so exp(score)
is exactly 1.0.  The softmax row then consists of a small band of
"interesting" values plus a uniform far field whose sums are known in
closed form.  We therefore compute only a narrow window of scores around
the diagonal on the PE and fold the far field in with exact host-side
constants:

    T0_i = (N - win) + sum_window exp(s)            (denominator)
    T1_i = C1_i + sum_window exp(s)*c + ws_i * sum_window exp(s)
    out_i = T1_i / T0_i - i

where C1_i = sum_all_j j - sum_window_i j (exact integers < 2^24, exact in
f32) and ws_i is the window start of row i's 64-row half-tile.  In-window
far entries have score exactly 0 (prior premultiplied in, 0 outside the
band) and contribute exp(0)=1, which the constants account for.

v2 layout: rows processed as 64-row halves packed two-per-partition-dim;
FOUR row tiles are batched per postprocessing pass ([P, 4*win]) to
amortize fixed per-op engine costs.  Projections use [P, 1024] PSUM
tiles (2 banks) so PSUM evictions are 4 big ops.  Work distribution:
ACT does k evictions + exp; DVE does q evictions + prior-mul + sum_e
reduces + the tail quad's fused mul-reduce (TTR) + combine; GpSimd does
the e*j multiplies and sum_ec reduces for the first 3 quads.  All input
DMA triggers are issued first (sync queue), PE warms up on junk matmuls
while they land.

Sharding: pure data-parallel over batch B=8 across the 8 cores; each core
holds the full (small) QK weights and computes its own [N] output row.
"""

import sys

sys.path.insert(0, "/opt/trn_rl_repo")

import ml_dtypes
import numpy as np

import concourse.bacc as bacc
import concourse.tile as tile
from concourse import mybir
from concourse.bass_utils import run_bass_kernel_spmd

B, N, D, MD = 8, 2048, 256, 128
NCORES = 8
P = 128
HR = P // 2  # 64-row half-tiles
NT = N // P  # 16 row tiles
NQUAD = NT // 4  # 4 postprocessing quads
DCH = D // P  # 2 contraction chunks for the projections
XCH = 512  # xt DMA chunk width (columns of x^T per chunk)
NXC = N // XCH  # 4 xt chunks
PW = 1024  # projection psum tile width (2 PSUM banks)
NPH = N // PW  # 2 projection halves
NJUNK = 6  # PE warmup matmuls
PI = 3.1415926  # matches reference
F32 = mybir.dt.float32
BF16 = mybir.dt.bfloat16

_cache = {}
# exposed for test harness profiling: (nc, in_maps)
last_run = None


def _plan_band(prior_mean, prior_std):
    """f32 prior over every offset, exactly as the reference computes it,
    and the band of offsets whose scores can round exp() away from 1.0."""
    d = np.arange(-(N - 1), N, dtype=np.float32)
    ps = np.float32(prior_std)
    pm = np.float32(prior_mean)
    prior = (
        np.float32(1.0)
        / ps
        / np.sqrt(np.float32(2.0) * np.float32(PI))
        * np.exp(np.float32(-0.5) * (d - pm) ** 2 / ps**2)
    ).astype(np.float32)
    # |score| <= |prior| * |q.k*scale| ; bound the latter by 1024 (actual
    # max is ~7 for these glorot inputs).  exp(x) rounds to 1.0f for
    # |x| < 2^-26; use 2^-27 for margin.
    sig = np.abs(prior) * 1024.0 >= 2.0**-27
    if not sig.any():
        dlo, dhi = 0, 0
    else:
        dlo = int(d[sig].min())
        dhi = int(d[sig].max())
    return prior, dlo, dhi


def _window_geometry(dlo, dhi):
    """Per-64-row-half window starts ws2[32] plus deduplicated per-quad
    prior patterns.  Pattern key for quad g (tiles 4g..4g+3) is the tuple
    of its eight half-window offsets relative to the quad's base row."""
    span = dhi - dlo
    win = HR + span + 1
    win = max(80, ((win + 15) // 16) * 16)
    assert win <= 128, f"prior band too wide for banded kernel: {dlo}..{dhi}"
    extra = win - (HR + span)
    ws2 = []
    for h in range(2 * NT):
        ws = min(max(h * HR + dlo - extra // 2, 0), N - win)
        lo_need = max(0, h * HR + dlo)
        hi_need = min(N - 1, h * HR + HR - 1 + dhi)
        assert ws <= lo_need and hi_need < ws + win, (h, ws, lo_need, hi_need)
        ws2.append(ws)
    quad_keys = []
    for g in range(NQUAD):
        base = 4 * P * g
        quad_keys.append(tuple(ws2[8 * g + i] - base for i in range(8)))
    key_vals = sorted(set(quad_keys))
    key_idx = [key_vals.index(k) for k in quad_keys]
    return win, ws2, key_vals, key_idx


def _build(win, ws2, key_idx, n_pat):
    nc = bacc.Bacc()

    QW = 4 * win  # postprocessing pass width
    assert QW <= 512

    # f32 consts: bq | bk | c1 | wsm | ii | j0quad
    O_BQ, O_BK = 0, 1
    O_C1 = 2
    O_WS = O_C1 + NT
    O_II = O_WS + NT
    O_J0 = O_II + NT
    CF = O_J0 + QW

    w2_d = nc.dram_tensor("w2", [P, 2 * DCH * MD], BF16, kind="ExternalInput")
    xt_d = nc.dram_tensor("xt", [NXC, P, DCH * XCH], BF16, kind="ExternalInput")
    ccf_d = nc.dram_tensor("ccf", [P, CF], F32, kind="ExternalInput")
    ccb_d = nc.dram_tensor("ccb", [P, n_pat * QW], BF16, kind="ExternalInput")
    y_d = nc.dram_tensor("y", [P, NT], F32, kind="ExternalOutput")

    with tile.TileContext(nc) as tc:
        with (
            tc.tile_pool(name="const", bufs=1) as const,
            tc.tile_pool(name="psum_proj", bufs=2, space="PSUM") as psum_proj,
            tc.tile_pool(name="psum_band", bufs=2, space="PSUM") as psum_band,
            tc.tile_pool(name="band_sp", bufs=2) as sp_pool,
            tc.tile_pool(name="band_e", bufs=2) as e_pool,
            tc.tile_pool(name="band_ej", bufs=2) as ej_pool,
            tc.tile_pool(name="ttr", bufs=1) as ttr_pool,
            tc.tile_pool(name="comb", bufs=1) as comb,
        ):
            # ---- input DMA triggers first, spread across the three DMA-
            # capable queues so descriptor generation isn't serialized:
            # sync (HWDGE): w2, xt0, xt1; gpsimd (SWDGE): xt2, xt3;
            # scalar (HWDGE): ccf, ccb.
            w2_s = const.tile([P, 2 * DCH * MD], BF16, tag="w2")
            nc.sync.dma_start(out=w2_s, in_=w2_d[:, :])
            xts = []
            for i in range(NXC):
                xt_tile = const.tile([P, DCH * XCH], BF16, tag=f"xt{i}")
                xts.append(xt_tile)
            nc.sync.dma_start(out=xts[0], in_=xt_d[0])
            nc.sync.dma_start(out=xts[1], in_=xt_d[1])
            nc.gpsimd.dma_start(out=xts[2], in_=xt_d[2])
            nc.gpsimd.dma_start(out=xts[3], in_=xt_d[3])
            ccf_s = const.tile([P, CF], F32, tag="ccf")
            nc.scalar.dma_start(out=ccf_s, in_=ccf_d[:, :])
            ccb_s = const.tile([P, n_pat * QW], BF16, tag="ccb")
            nc.scalar.dma_start(out=ccb_s, in_=ccb_d[:, :])

            # ---- engine warmups (run while DMAs are in flight) ----
            # PE: junk matmuls keep the PE busy until the input DMAs land,
            # flipping the HAM clock gate to 8/8 before the real matmuls.
            # ACT: one tiny Exp pulls the 1.3us ACT_TABLE_LOAD off the
            # critical path.
            wtile = const.tile([P, XCH], BF16, tag="warm_w")
            nc.vector.memset(wtile, 0.0)
            wact_in = const.tile([P, 1], F32, tag="warm_a")
            nc.vector.memset(wact_in, 0.0)
            for _ in range(NJUNK):
                wps = psum_proj.tile([P, PW], F32, tag="proj")
                nc.tensor.matmul(
                    wps[:, :XCH],
                    lhsT=wtile[:, :P],
                    rhs=wtile[:, :XCH],
                    start=True,
                    stop=True,
                )
            wact_out = const.tile([P, 1], F32, tag="warm_ao")
            nc.scalar.activation(
                out=wact_out, in_=wact_in, func=mybir.ActivationFunctionType.Exp
            )

            qT = const.tile([P, N], BF16, tag="qT")
            kT = const.tile([P, N], BF16, tag="kT")
            sum_e = const.tile([P, NT], F32, tag="sum_e")
            sum_ec = const.tile([P, NT], F32, tag="sum_ec")
            outv2 = comb.tile([P, NT], F32, tag="outv2")

            # ---- projections: psum tile = [P, 1024] (2 banks, 4 matmuls,
            # c-outer so consecutive matmuls share LDWEIGHTS), evicted in
            # one [P, 1024] op.  q evictions on DVE (idle early), k on ACT.
            proj_ps = {}

            def emit_proj_mms(pj, h):  # pj 0=q 1=k, h = 1024-col half
                ps_t = psum_proj.tile([P, PW], F32, tag="proj")
                proj_ps[(pj, h)] = ps_t
                for c in range(DCH):
                    for nn in range(PW // XCH):
                        xc = (PW // XCH) * h + nn
                        nc.tensor.matmul(
                            ps_t[:, nn * XCH : (nn + 1) * XCH],
                            lhsT=w2_s[:, (2 * pj + c) * MD : (2 * pj + c + 1) * MD],
                            rhs=xts[xc][:, c * XCH : (c + 1) * XCH],
                            start=(c == 0),
                            stop=(c == DCH - 1),
                        )

            def emit_evict(pj, h, eng):
                dstT = (qT, kT)[pj]
                b_s = ccf_s[:, O_BQ + pj : O_BQ + pj + 1]
                ps_t = proj_ps[(pj, h)]
                if eng is nc.vector:
                    nc.vector.tensor_scalar_add(
                        dstT[:, h * PW : (h + 1) * PW], ps_t, b_s
                    )
                else:
                    nc.scalar.activation(
                        out=dstT[:, h * PW : (h + 1) * PW],
                        in_=ps_t,
                        func=mybir.ActivationFunctionType.Identity,
                        bias=b_s,
                        scale=1.0,
                    )

            # PE order: junk, q0, k0, q1, k1, band quads 0..3
            emit_proj_mms(0, 0)
            emit_proj_mms(1, 0)
            emit_proj_mms(0, 1)
            emit_proj_mms(1, 1)

            # evictions (cross-queue; Tile sems order them after the MMs)
            emit_evict(0, 0, nc.vector)  # DVE, idle until postproc starts
            emit_evict(1, 0, nc.scalar)  # ACT
            emit_evict(0, 1, nc.vector)  # DVE
            emit_evict(1, 1, nc.scalar)  # ACT

            # ---- band quads: tiles 4g..4g+3 share one [P, 4*win] pass ----
            e_ts = {}

            def emit_band_mms(g):
                ps_s = psum_band.tile([P, 512], F32, tag="band")
                for tq in range(4):
                    t = 4 * g + tq
                    for hb in range(2):
                        ws = ws2[2 * t + hb]
                        nc.tensor.matmul(
                            ps_s[
                                hb * HR : (hb + 1) * HR, tq * win : (tq + 1) * win
                            ],
                            lhsT=qT[:, t * P + hb * HR : t * P + (hb + 1) * HR],
                            rhs=kT[:, ws : ws + win],
                            start=True,
                            stop=True,
                        )
                return ps_s

            band_ps = [None] * NQUAD

            def emit_mul(g):
                ps_s = band_ps[g]
                oi = key_idx[g]
                sp_t = sp_pool.tile([P, QW], F32, tag="sp")
                nc.vector.tensor_mul(
                    sp_t, ps_s[:, :QW], ccb_s[:, oi * QW : (oi + 1) * QW]
                )
                return sp_t

            def emit_exp(g, sp_t):
                e_t = e_pool.tile([P, QW], F32, tag="e")
                nc.scalar.activation(
                    out=e_t, in_=sp_t, func=mybir.ActivationFunctionType.Exp
                )
                e_ts[g] = e_t

            def emit_exp_accum(g, sp_t):
                # per-tile exp with fused row-sum: sum_e comes for free on
                # ACT, taking that reduce off the DVE
                e_t = e_pool.tile([P, QW], F32, tag="e")
                for tq in range(4):
                    t = 4 * g + tq
                    nc.scalar.activation(
                        out=e_t[:, tq * win : (tq + 1) * win],
                        in_=sp_t[:, tq * win : (tq + 1) * win],
                        func=mybir.ActivationFunctionType.Exp,
                        accum_out=sum_e[:, t : t + 1],
                    )
                e_ts[g] = e_t

            def emit_re(g):  # sum_e for quad g on DVE
                nc.vector.tensor_reduce(
                    out=sum_e[:, 4 * g : 4 * g + 4],
                    in_=e_ts[g][:].rearrange("p (t w) -> p t w", w=win),
                    axis=mybir.AxisListType.X,
                    op=mybir.AluOpType.add,
                )

            ej_ts = {}

            def emit_ej(g):  # e*j0 for quad g on GpSimd
                ej_t = ej_pool.tile([P, QW], F32, tag="ej")
                nc.gpsimd.tensor_mul(ej_t, e_ts[g], ccf_s[:, O_J0 : O_J0 + QW])
                ej_ts[g] = ej_t

            def emit_rec(g):  # sum_ec for quad g on DVE
                nc.vector.tensor_reduce(
                    out=sum_ec[:, 4 * g : 4 * g + 4],
                    in_=ej_ts[g][:].rearrange("p (t w) -> p t w", w=win),
                    axis=mybir.AxisListType.X,
                    op=mybir.AluOpType.add,
                )

            def emit_ttr(g):  # fused e*j0 + sum_ec for quad g on DVE (tail)
                for tq in range(4):
                    t = 4 * g + tq
                    scr = ttr_pool.tile([P, win], F32, tag="scr")
                    nc.vector.tensor_tensor_reduce(
                        out=scr,
                        in0=e_ts[g][:, tq * win : (tq + 1) * win],
                        in1=ccf_s[:, O_J0 : O_J0 + win],
                        scale=1.0,
                        scalar=0.0,
                        op0=mybir.AluOpType.mult,
                        op1=mybir.AluOpType.add,
                        accum_out=sum_ec[:, t : t + 1],
                    )

            # ---- combine: out = (c1 + sum_ec + ws*sum_e)/(N-win+sum_e) - i
            c1_s = ccf_s[:, O_C1 : O_C1 + NT]
            ws_s = ccf_s[:, O_WS : O_WS + NT]
            ii_s = ccf_s[:, O_II : O_II + NT]

            def emit_combine(sl):
                w = sl.stop - sl.start
                t0 = comb.tile([P, w], F32, tag="t0")
                nc.vector.tensor_scalar_add(t0, sum_e[:, sl], float(N - win))
                rec = comb.tile([P, w], F32, tag="rec")
                nc.vector.reciprocal(rec, t0)
                tmp = comb.tile([P, w], F32, tag="tmp")
                nc.vector.tensor_mul(tmp, ws_s[:, sl], sum_e[:, sl])
                num = comb.tile([P, w], F32, tag="num")
                nc.vector.tensor_add(num, c1_s[:, sl], sum_ec[:, sl])
                num2 = comb.tile([P, w], F32, tag="num2")
                nc.vector.tensor_add(num2, num, tmp)
                outv = comb.tile([P, w], F32, tag="outv")
                nc.vector.tensor_mul(outv, num2, rec)
                nc.vector.tensor_sub(outv2[:, sl], outv, ii_s[:, sl])

            def emit_comb_recip(sl):
                # tiny DVE part of the GpSimd combine: 1/(N-win+sum_e)
                # (divide is not a legal Pool-engine opcode)
                w = sl.stop - sl.start
                t0 = comb.tile([P, w], F32, tag="gt0")
                nc.vector.tensor_scalar_add(t0, sum_e[:, sl], float(N - win))
                rec = comb.tile([P, w], F32, tag="grec")
                nc.vector.reciprocal(rec, t0)
                return rec

            def emit_combine_gp(sl, rec):
                # first-half combine on GpSimd, keeping the DVE free for
                # the tail quads
                w = sl.stop - sl.start
                tmp = comb.tile([P, w], F32, tag="gtmp")
                nc.gpsimd.tensor_mul(tmp, ws_s[:, sl], sum_e[:, sl])
                num = comb.tile([P, w], F32, tag="gnum")
                nc.gpsimd.tensor_add(num, c1_s[:, sl], sum_ec[:, sl])
                num2 = comb.tile([P, w], F32, tag="gnum2")
                nc.gpsimd.tensor_add(num2, num, tmp)
                outv = comb.tile([P, w], F32, tag="goutv")
                nc.gpsimd.tensor_mul(outv, num2, rec)
                nc.gpsimd.tensor_sub(outv2[:, sl], outv, ii_s[:, sl])

            # band matmuls all after the projections on the PE queue
            for g in range(NQUAD):
                band_ps[g] = emit_band_mms(g)

            # postprocessing:
            #   DVE:    [mul0, mul1, rec0, mul2, re2, rec1, mul3, re3,
            #            rec2, rec3, combB]
            #   ACT:    [exp0(4x accum), exp1(4x accum), exp2, exp3]
            #   GpSimd: [ej0, ej1, ej2, ej3, combA]
            sp0 = emit_mul(0)
            emit_exp_accum(0, sp0)
            emit_ej(0)
            sp1 = emit_mul(1)
            emit_exp_accum(1, sp1)
            emit_rec(0)
            emit_ej(1)
            sp2 = emit_mul(2)
            emit_exp(2, sp2)
            emit_re(2)
            emit_rec(1)
            recA = emit_comb_recip(slice(0, 8))
            emit_ej(2)
            sp3 = emit_mul(3)
            emit_exp(3, sp3)
            emit_re(3)
            emit_rec(2)
            emit_ej(3)
            emit_rec(3)
            emit_combine_gp(slice(0, 8), recA)
            nc.sync.dma_start(out=y_d[:, 0:8], in_=outv2[:, 0:8])
            emit_combine(slice(8, NT))
            nc.sync.dma_start(out=y_d[:, 8:NT], in_=outv2[:, 8:NT])

    nc.finalize()
    return nc


def kernel(x, Wq, bq, Wk, bk, prior_mean, prior_std):
    global last_run
    x = np.asarray(x, dtype=np.float32)
    Wq = np.asarray(Wq, dtype=np.float32)
    Wk = np.asarray(Wk, dtype=np.float32)
    bq = np.asarray(bq, dtype=np.float32)
    bk = np.asarray(bk, dtype=np.float32)

    prior, dlo, dhi = _plan_band(
        float(np.asarray(prior_mean)[0]), float(np.asarray(prior_std)[0])
    )
    win, ws2, key_vals, key_idx = _window_geometry(dlo, dhi)
    n_pat = len(key_vals)
    QW = 4 * win

    key = (win, tuple(ws2), tuple(key_idx))
    if key not in _cache:
        _cache[key] = _build(win, ws2, key_idx, n_pat)
    nc = _cache[key]

    bf = ml_dtypes.bfloat16
    scale = np.float32(MD**-0.5)

    # prior*scale quad patterns: [P, 4*win] per distinct 8-offset key.
    # value[p, tq*win + c] = prior[c + rel_ws[tq, hb] - 128*tq - p] * scale
    # where hb selects by partition half (p >= 64).
    p_idx = np.arange(P)[:, None]
    c_idx = np.arange(win)[None, :]
    pmat = np.zeros((P, n_pat * QW), np.float32)
    for ki, rel in enumerate(key_vals):
        for tq in range(4):
            relcol = np.where(np.arange(P) < HR, rel[2 * tq], rel[2 * tq + 1])[
                :, None
            ]
            dm = c_idx + relcol - 128 * tq - p_idx
            pmat[:, ki * QW + tq * win : ki * QW + (tq + 1) * win] = np.where(
                (dm >= dlo) & (dm <= dhi), prior[dm + N - 1] * scale, np.float32(0.0)
            ).astype(np.float32)

    sumj_all = float(N * (N - 1) // 2)
    c1 = np.zeros((P, NT), np.float32)
    wsm = np.zeros((P, NT), np.float32)
    ii = np.zeros((P, NT), np.float32)
    half_sel = np.arange(P) >= HR
    for t in range(NT):
        wsa, wsb = ws2[2 * t], ws2[2 * t + 1]
        wsv = np.where(half_sel, float(wsb), float(wsa))
        c1[:, t] = sumj_all - (win * wsv + win * (win - 1) // 2)
        wsm[:, t] = wsv
        ii[:, t] = t * P + np.arange(P)

    # f32 consts: bq | bk | c1 | wsm | ii | j0quad
    j0q = np.broadcast_to(
        np.tile(np.arange(win, dtype=np.float32), 4), (P, QW)
    )
    ccf = np.ascontiguousarray(
        np.concatenate(
            [bq.reshape(P, 1), bk.reshape(P, 1), c1, wsm, ii, j0q], axis=1
        ).astype(np.float32)
    )
    ccb = np.ascontiguousarray(pmat.astype(bf))

    # weights: wq chunks then wk chunks, [P, 4*MD]
    wq_h = Wq.reshape(DCH, P, MD).transpose(1, 0, 2).reshape(P, DCH * MD)
    wk_h = Wk.reshape(DCH, P, MD).transpose(1, 0, 2).reshape(P, DCH * MD)
    w2_h = np.ascontiguousarray(np.concatenate([wq_h, wk_h], axis=1)).astype(bf)

    in_maps = []
    for core in range(NCORES):
        xb = x[core]  # [N, D]
        # xt[i, p, c*512 + j] = x[i*512 + j, c*128 + p]
        xt_h = np.ascontiguousarray(
            xb.T.reshape(DCH, P, NXC, XCH)
            .transpose(2, 1, 0, 3)
            .reshape(NXC, P, DCH * XCH)
        ).astype(bf)
        in_maps.append({"xt": xt_h, "w2": w2_h, "ccf": ccf, "ccb": ccb})

    res = run_bass_kernel_spmd(nc, in_maps, list(range(NCORES)))
    last_run = (nc, in_maps)
    # y[p, t] = out[128t + p]  ->  out = y.T.flatten()
    out = np.stack(
        [res.results[c]["y"].T.reshape(-1) for c in range(NCORES)], axis=0
    )
    return out.astype(np.float32)


# revision 19
# speedup vs baseline: 1.1881x; 1.1881x over previous
"""Trainium2 Bass kernel for nn_DistanceLayer (gaussian-prior distance attention).

Math: out[b,i] = sum_j softmax_j(q_i.k_j * MD^-0.5 * prior(j-i))[j] * (j-i)

Key observation: the gaussian prior (std=1) underflows so fast in f32 that
for |j-i| outside a small band the f32 score is exactly 0, so exp(score)
is exactly 1.0.  The softmax row then consists of a small band of
"interesting" values plus a uniform far field whose sums are known in
closed form.  We therefore compute only a narrow window of scores around
the diagonal on the PE and fold the far field in with exact host-side
constants:

    T0_i = (N - win) + sum_window exp(s)            (denominator)
    T1_i = C1_i + sum_window exp(s)*c + ws_i * sum_window exp(s)
    out_i = T1_i / T0_i - i

where C1_i = sum_all_j j - sum_window_i j (exact integers < 2^24, exact in
f32) and ws_i is the window start of row i's 64-row half-tile.  In-window
far entries have score exactly 0 (prior premultiplied in, 0 outside the
band) and contribute exp(0)=1, which the constants account for.

v2 layout: rows processed as 64-row halves packed two-per-partition-dim;
FOUR row tiles are batched per postprocessing pass ([P, 4*win]) to
amortize fixed per-op engine costs.  Projections use [P, 1024] PSUM
tiles (2 banks) so PSUM evictions are 4 big ops.  Work distribution:
ACT does k evictions + exp; DVE does q evictions + prior-mul + sum_e
reduces + the tail quad's fused mul-reduce (TTR) + combine; GpSimd does
the e*j multiplies and sum_ec reduces for the first 3 quads.  All input
DMA triggers are issued first (sync queue), PE warms up on junk matmuls
while they land.

Sharding: pure data-parallel over batch B=8 across the 8 cores; each core
holds the full (small) QK weights and computes its own [N] output row.
"""

import sys

sys.path.insert(0, "/opt/trn_rl_repo")

import ml_dtypes
import numpy as np

import concourse.bacc as bacc
import concourse.tile as tile
from concourse import mybir
from concourse.bass_utils import run_bass_kernel_spmd

B, N, D, MD = 8, 2048, 256, 128
NCORES = 8
P = 128
HR = P // 2  # 64-row half-tiles
NT = N // P  # 16 row tiles
NQUAD = NT // 4  # 4 postprocessing quads
DCH = D // P  # 2 contraction chunks for the projections
XCH = 512  # xt DMA chunk width (columns of x^T per chunk)
NXC = N // XCH  # 4 xt chunks
PW = 1024  # projection psum tile width (2 PSUM banks)
NPH = N // PW  # 2 projection halves
NJUNK = 5  # PE warmup matmuls
PI = 3.1415926  # matches reference
F32 = mybir.dt.float32
BF16 = mybir.dt.bfloat16
FP8 = mybir.dt.float8e4

_cache = {}
# exposed for test harness profiling: (nc, in_maps)
last_run = None


def _plan_band(prior_mean, prior_std):
    """f32 prior over every offset, exactly as the reference computes it,
    and the band of offsets whose scores can round exp() away from 1.0."""
    d = np.arange(-(N - 1), N, dtype=np.float32)
    ps = np.float32(prior_std)
    pm = np.float32(prior_mean)
    prior = (
        np.float32(1.0)
        / ps
        / np.sqrt(np.float32(2.0) * np.float32(PI))
        * np.exp(np.float32(-0.5) * (d - pm) ** 2 / ps**2)
    ).astype(np.float32)
    # |score| <= |prior| * |q.k*scale| ; bound the latter by 1024 (actual
    # max is ~7 for these glorot inputs).  exp(x) rounds to 1.0f for
    # |x| < 2^-26; use 2^-27 for margin.
    sig = np.abs(prior) * 1024.0 >= 2.0**-27
    if not sig.any():
        dlo, dhi = 0, 0
    else:
        dlo = int(d[sig].min())
        dhi = int(d[sig].max())
    return prior, dlo, dhi


def _window_geometry(dlo, dhi):
    """Per-64-row-half window starts ws2[32] plus deduplicated per-quad
    prior patterns.  Pattern key for quad g (tiles 4g..4g+3) is the tuple
    of its eight half-window offsets relative to the quad's base row."""
    span = dhi - dlo
    win = HR + span + 1
    win = max(80, ((win + 15) // 16) * 16)
    assert win <= 128, f"prior band too wide for banded kernel: {dlo}..{dhi}"
    extra = win - (HR + span)
    ws2 = []
    for h in range(2 * NT):
        ws = min(max(h * HR + dlo - extra // 2, 0), N - win)
        lo_need = max(0, h * HR + dlo)
        hi_need = min(N - 1, h * HR + HR - 1 + dhi)
        assert ws <= lo_need and hi_need < ws + win, (h, ws, lo_need, hi_need)
        ws2.append(ws)
    quad_keys = []
    for g in range(NQUAD):
        base = 4 * P * g
        quad_keys.append(tuple(ws2[8 * g + i] - base for i in range(8)))
    key_vals = sorted(set(quad_keys))
    key_idx = [key_vals.index(k) for k in quad_keys]
    return win, ws2, key_vals, key_idx


def _build(win, ws2, key_idx, n_pat):
    nc = bacc.Bacc()

    QW = 4 * win  # postprocessing pass width
    assert QW <= 512

    # f32 consts: bq | bk | c1 | wsm | ii | j0quad
    O_BQ, O_BK = 0, 1
    O_C1 = 2
    O_WS = O_C1 + NT
    O_II = O_WS + NT
    O_J0 = O_II + NT
    CF = O_J0 + QW

    w2_d = nc.dram_tensor("w2", [P, 2 * DCH, MD], FP8, kind="ExternalInput")
    xt_d = nc.dram_tensor("xt", [NXC, P, DCH, XCH], FP8, kind="ExternalInput")
    ccf_d = nc.dram_tensor("ccf", [P, CF], F32, kind="ExternalInput")
    ccb_d = nc.dram_tensor("ccb", [P, n_pat * QW], BF16, kind="ExternalInput")
    y_d = nc.dram_tensor("y", [P, NT], F32, kind="ExternalOutput")

    with tile.TileContext(nc) as tc:
        with (
            tc.tile_pool(name="const", bufs=1) as const,
            tc.tile_pool(name="psum_proj", bufs=2, space="PSUM") as psum_proj,
            tc.tile_pool(name="psum_band", bufs=2, space="PSUM") as psum_band,
            tc.tile_pool(name="band_sp", bufs=2) as sp_pool,
            tc.tile_pool(name="band_e", bufs=2) as e_pool,
            tc.tile_pool(name="band_ej", bufs=2) as ej_pool,
            tc.tile_pool(name="ttr", bufs=1) as ttr_pool,
            tc.tile_pool(name="comb", bufs=1) as comb,
        ):
            # ---- input DMA triggers first.  All the PE-feeding tensors go
            # on the sync queue IN PRIORITY ORDER: a single HWDGE ring
            # drains near-FIFO, so earlier triggers finish their transfers
            # first (the front is DMA-bandwidth-bound, so completion order
            # is what matters).  Consts ride the scalar ring.
            w2_s = const.tile([P, 2 * DCH, MD], FP8, tag="w2")
            nc.sync.dma_start(out=w2_s, in_=w2_d[:, :, :])
            xts = []
            for i in range(NXC):
                xt_tile = const.tile([P, DCH, XCH], FP8, tag=f"xt{i}")
                xts.append(xt_tile)
            for i in range(NXC):
                nc.sync.dma_start(out=xts[i], in_=xt_d[i])
            ccf_s = const.tile([P, CF], F32, tag="ccf")
            nc.scalar.dma_start(out=ccf_s, in_=ccf_d[:, :])
            ccb_s = const.tile([P, n_pat * QW], BF16, tag="ccb")
            nc.scalar.dma_start(out=ccb_s, in_=ccb_d[:, :])

            # ---- engine warmups (run while DMAs are in flight) ----
            # PE: junk matmuls keep the PE busy until the input DMAs land,
            # flipping the HAM clock gate to 8/8 before the real matmuls.
            # ACT: one tiny Exp pulls the 1.3us ACT_TABLE_LOAD off the
            # critical path.
            wtile = const.tile([P, XCH], BF16, tag="warm_w")
            nc.vector.memset(wtile, 0.0)
            wact_in = const.tile([P, 1], F32, tag="warm_a")
            nc.vector.memset(wact_in, 0.0)
            for _ in range(NJUNK):
                wps = psum_proj.tile([P, PW], F32, tag="proj")
                nc.tensor.matmul(
                    wps[:, :XCH],
                    lhsT=wtile[:, :P],
                    rhs=wtile[:, :XCH],
                    start=True,
                    stop=True,
                )
            wact_out = const.tile([P, 1], F32, tag="warm_ao")
            nc.scalar.activation(
                out=wact_out, in_=wact_in, func=mybir.ActivationFunctionType.Exp
            )

            qT = const.tile([P, N], BF16, tag="qT")
            kT = const.tile([P, N], BF16, tag="kT")
            sum_e = const.tile([P, NT], F32, tag="sum_e")
            sum_ec = const.tile([P, NT], F32, tag="sum_ec")
            outv2 = comb.tile([P, NT], F32, tag="outv2")

            # ---- projections: fp8 DoubleRow matmuls — the whole D=256
            # contraction (2 subtiles, 2 fp8 weights per PE cell) in ONE
            # matmul per 512-col chunk.  psum tile = [P, 1024] (2 banks),
            # evicted in one [P, 1024] op (except k1, split in two so the
            # band quads unblock sooner).  q evictions on DVE (idle
            # early), k on ACT.
            proj_ps = {}

            def emit_proj_mms(pj, h):  # pj 0=q 1=k, h = 1024-col half
                ps_t = psum_proj.tile([P, PW], F32, tag="proj")
                proj_ps[(pj, h)] = ps_t
                for nn in range(PW // XCH):
                    xc = (PW // XCH) * h + nn
                    nc.tensor.matmul(
                        ps_t[:, nn * XCH : (nn + 1) * XCH],
                        lhsT=w2_s[:, 2 * pj : 2 * pj + DCH, :],
                        rhs=xts[xc][:, :, :],
                        start=True,
                        stop=True,
                        perf_mode=mybir.MatmulPerfMode.DoubleRow,
                    )

            def emit_evict(pj, h, eng, part=None):
                dstT = (qT, kT)[pj]
                b_s = ccf_s[:, O_BQ + pj : O_BQ + pj + 1]
                ps_t = proj_ps[(pj, h)]
                lo, hi = (0, PW) if part is None else (part * XCH, (part + 1) * XCH)
                if eng is nc.vector:
                    nc.vector.tensor_scalar_add(
                        dstT[:, h * PW + lo : h * PW + hi], ps_t[:, lo:hi], b_s
                    )
                else:
                    nc.scalar.activation(
                        out=dstT[:, h * PW + lo : h * PW + hi],
                        in_=ps_t[:, lo:hi],
                        func=mybir.ActivationFunctionType.Identity,
                        bias=b_s,
                        scale=1.0,
                    )

            # PE order: junk, q0, k0, q1, k1, band quads 0..3
            emit_proj_mms(0, 0)
            emit_proj_mms(1, 0)
            emit_proj_mms(0, 1)
            emit_proj_mms(1, 1)

            # evictions (cross-queue; Tile sems order them after the MMs)
            emit_evict(0, 0, nc.vector)  # DVE, idle until postproc starts
            emit_evict(1, 0, nc.scalar)  # ACT
            emit_evict(0, 1, nc.vector)  # DVE
            emit_evict(1, 1, nc.scalar, part=0)  # ACT; unblocks band quads 1-2
            emit_evict(1, 1, nc.scalar, part=1)  # ACT

            # ---- band quads: tiles 4g..4g+3 share one [P, 4*win] pass ----
            e_ts = {}

            def emit_band_mms(g):
                ps_s = psum_band.tile([P, 512], F32, tag="band")
                for tq in range(4):
                    t = 4 * g + tq
                    for hb in range(2):
                        ws = ws2[2 * t + hb]
                        nc.tensor.matmul(
                            ps_s[
                                hb * HR : (hb + 1) * HR, tq * win : (tq + 1) * win
                            ],
                            lhsT=qT[:, t * P + hb * HR : t * P + (hb + 1) * HR],
                            rhs=kT[:, ws : ws + win],
                            start=True,
                            stop=True,
                        )
                return ps_s

            band_ps = [None] * NQUAD

            def emit_mul(g):
                ps_s = band_ps[g]
                oi = key_idx[g]
                sp_t = sp_pool.tile([P, QW], F32, tag="sp")
                nc.vector.tensor_mul(
                    sp_t, ps_s[:, :QW], ccb_s[:, oi * QW : (oi + 1) * QW]
                )
                return sp_t

            def emit_exp(g, sp_t):
                e_t = e_pool.tile([P, QW], F32, tag="e")
                nc.scalar.activation(
                    out=e_t, in_=sp_t, func=mybir.ActivationFunctionType.Exp
                )
                e_ts[g] = e_t

            def emit_exp_accum(g, sp_t):
                # per-tile exp with fused row-sum: sum_e comes for free on
                # ACT, taking that reduce off the DVE
                e_t = e_pool.tile([P, QW], F32, tag="e")
                for tq in range(4):
                    t = 4 * g + tq
                    nc.scalar.activation(
                        out=e_t[:, tq * win : (tq + 1) * win],
                        in_=sp_t[:, tq * win : (tq + 1) * win],
                        func=mybir.ActivationFunctionType.Exp,
                        accum_out=sum_e[:, t : t + 1],
                    )
                e_ts[g] = e_t

            def emit_re(g):  # sum_e for quad g on DVE
                nc.vector.tensor_reduce(
                    out=sum_e[:, 4 * g : 4 * g + 4],
                    in_=e_ts[g][:].rearrange("p (t w) -> p t w", w=win),
                    axis=mybir.AxisListType.X,
                    op=mybir.AluOpType.add,
                )

            ej_ts = {}

            def emit_ej(g):  # e*j0 for quad g on GpSimd
                ej_t = ej_pool.tile([P, QW], F32, tag="ej")
                nc.gpsimd.tensor_mul(ej_t, e_ts[g], ccf_s[:, O_J0 : O_J0 + QW])
                ej_ts[g] = ej_t

            def emit_rec(g):  # sum_ec for quad g on DVE
                nc.vector.tensor_reduce(
                    out=sum_ec[:, 4 * g : 4 * g + 4],
                    in_=ej_ts[g][:].rearrange("p (t w) -> p t w", w=win),
                    axis=mybir.AxisListType.X,
                    op=mybir.AluOpType.add,
                )

            def emit_ttr(g):  # fused e*j0 + sum_ec for quad g on DVE (tail)
                for tq in range(4):
                    t = 4 * g + tq
                    scr = ttr_pool.tile([P, win], F32, tag="scr")
                    nc.vector.tensor_tensor_reduce(
                        out=scr,
                        in0=e_ts[g][:, tq * win : (tq + 1) * win],
                        in1=ccf_s[:, O_J0 : O_J0 + win],
                        scale=1.0,
                        scalar=0.0,
                        op0=mybir.AluOpType.mult,
                        op1=mybir.AluOpType.add,
                        accum_out=sum_ec[:, t : t + 1],
                    )

            # ---- combine: out = (c1 + sum_ec + ws*sum_e)/(N-win+sum_e) - i
            c1_s = ccf_s[:, O_C1 : O_C1 + NT]
            ws_s = ccf_s[:, O_WS : O_WS + NT]
            ii_s = ccf_s[:, O_II : O_II + NT]

            def emit_combine(sl):
                w = sl.stop - sl.start
                t0 = comb.tile([P, w], F32, tag="t0")
                nc.vector.tensor_scalar_add(t0, sum_e[:, sl], float(N - win))
                rec = comb.tile([P, w], F32, tag="rec")
                nc.vector.reciprocal(rec, t0)
                tmp = comb.tile([P, w], F32, tag="tmp")
                nc.vector.tensor_mul(tmp, ws_s[:, sl], sum_e[:, sl])
                num = comb.tile([P, w], F32, tag="num")
                nc.vector.tensor_add(num, c1_s[:, sl], sum_ec[:, sl])
                num2 = comb.tile([P, w], F32, tag="num2")
                nc.vector.tensor_add(num2, num, tmp)
                outv = comb.tile([P, w], F32, tag="outv")
                nc.vector.tensor_mul(outv, num2, rec)
                nc.vector.tensor_sub(outv2[:, sl], outv, ii_s[:, sl])

            def emit_comb_recip(sl):
                # tiny DVE part of the GpSimd combine: 1/(N-win+sum_e)
                # (divide is not a legal Pool-engine opcode)
                w = sl.stop - sl.start
                t0 = comb.tile([P, w], F32, tag="gt0")
                nc.vector.tensor_scalar_add(t0, sum_e[:, sl], float(N - win))
                rec = comb.tile([P, w], F32, tag="grec")
                nc.vector.reciprocal(rec, t0)
                return rec

            def emit_combine_gp(sl, rec):
                # first-half combine on GpSimd, keeping the DVE free for
                # the tail quads
                w = sl.stop - sl.start
                tmp = comb.tile([P, w], F32, tag="gtmp")
                nc.gpsimd.tensor_mul(tmp, ws_s[:, sl], sum_e[:, sl])
                num = comb.tile([P, w], F32, tag="gnum")
                nc.gpsimd.tensor_add(num, c1_s[:, sl], sum_ec[:, sl])
                num2 = comb.tile([P, w], F32, tag="gnum2")
                nc.gpsimd.tensor_add(num2, num, tmp)
                outv = comb.tile([P, w], F32, tag="goutv")
                nc.gpsimd.tensor_mul(outv, num2, rec)
                nc.gpsimd.tensor_sub(outv2[:, sl], outv, ii_s[:, sl])

            # band matmuls all after the projections on the PE queue
            for g in range(NQUAD):
                band_ps[g] = emit_band_mms(g)

            # postprocessing:
            #   DVE:    [mul0, re0, mul1, re1, rec0, mul2, re2, rec1,
            #            recipA, mul3, re3, rec2, rec3, combB]
            #   ACT:    [exp0..exp3]
            #   GpSimd: [ej0, ej1, ej2, ej3, combA]
            sp0 = emit_mul(0)
            emit_exp(0, sp0)
            emit_re(0)
            emit_ej(0)
            sp1 = emit_mul(1)
            emit_exp(1, sp1)
            emit_re(1)
            emit_rec(0)
            emit_ej(1)
            sp2 = emit_mul(2)
            emit_exp(2, sp2)
            emit_re(2)
            emit_rec(1)
            recA = emit_comb_recip(slice(0, 8))
            emit_ej(2)
            sp3 = emit_mul(3)
            emit_exp(3, sp3)
            emit_re(3)
            emit_rec(2)
            emit_ej(3)
            emit_rec(3)
            emit_combine_gp(slice(0, 8), recA)
            nc.sync.dma_start(out=y_d[:, 0:8], in_=outv2[:, 0:8])
            emit_combine(slice(8, NT))
            nc.sync.dma_start(out=y_d[:, 8:NT], in_=outv2[:, 8:NT])

    nc.finalize()
    return nc


def kernel(x, Wq, bq, Wk, bk, prior_mean, prior_std):
    global last_run
    x = np.asarray(x, dtype=np.float32)
    Wq = np.asarray(Wq, dtype=np.float32)
    Wk = np.asarray(Wk, dtype=np.float32)
    bq = np.asarray(bq, dtype=np.float32)
    bk = np.asarray(bk, dtype=np.float32)

    prior, dlo, dhi = _plan_band(
        float(np.asarray(prior_mean)[0]), float(np.asarray(prior_std)[0])
    )
    win, ws2, key_vals, key_idx = _window_geometry(dlo, dhi)
    n_pat = len(key_vals)
    QW = 4 * win

    key = (win, tuple(ws2), tuple(key_idx))
    if key not in _cache:
        _cache[key] = _build(win, ws2, key_idx, n_pat)
    nc = _cache[key]

    bf = ml_dtypes.bfloat16
    scale = np.float32(MD**-0.5)

    # prior*scale quad patterns: [P, 4*win] per distinct 8-offset key.
    # value[p, tq*win + c] = prior[c + rel_ws[tq, hb] - 128*tq - p] * scale
    # where hb selects by partition half (p >= 64).
    p_idx = np.arange(P)[:, None]
    c_idx = np.arange(win)[None, :]
    pmat = np.zeros((P, n_pat * QW), np.float32)
    for ki, rel in enumerate(key_vals):
        for tq in range(4):
            relcol = np.where(np.arange(P) < HR, rel[2 * tq], rel[2 * tq + 1])[
                :, None
            ]
            dm = c_idx + relcol - 128 * tq - p_idx
            pmat[:, ki * QW + tq * win : ki * QW + (tq + 1) * win] = np.where(
                (dm >= dlo) & (dm <= dhi), prior[dm + N - 1] * scale, np.float32(0.0)
            ).astype(np.float32)

    sumj_all = float(N * (N - 1) // 2)
    c1 = np.zeros((P, NT), np.float32)
    wsm = np.zeros((P, NT), np.float32)
    ii = np.zeros((P, NT), np.float32)
    half_sel = np.arange(P) >= HR
    for t in range(NT):
        wsa, wsb = ws2[2 * t], ws2[2 * t + 1]
        wsv = np.where(half_sel, float(wsb), float(wsa))
        c1[:, t] = sumj_all - (win * wsv + win * (win - 1) // 2)
        wsm[:, t] = wsv
        ii[:, t] = t * P + np.arange(P)

    # f32 consts: bq | bk | c1 | wsm | ii | j0quad
    j0q = np.broadcast_to(
        np.tile(np.arange(win, dtype=np.float32), 4), (P, QW)
    )
    ccf = np.ascontiguousarray(
        np.concatenate(
            [bq.reshape(P, 1), bk.reshape(P, 1), c1, wsm, ii, j0q], axis=1
        ).astype(np.float32)
    )
    ccb = np.ascontiguousarray(pmat.astype(bf))

    # weights: wq chunks then wk chunks, [P, 2*DCH, MD] fp8
    f8 = ml_dtypes.float8_e4m3fn
    wq_h = Wq.reshape(DCH, P, MD).transpose(1, 0, 2)
    wk_h = Wk.reshape(DCH, P, MD).transpose(1, 0, 2)
    w2_h = np.ascontiguousarray(
        np.concatenate([wq_h, wk_h], axis=1)
    ).astype(f8)

    in_maps = []
    for core in range(NCORES):
        xb = x[core]  # [N, D]
        # xt[i, p, c, j] = x[i*512 + j, c*128 + p]
        xt_h = np.ascontiguousarray(
            xb.T.reshape(DCH, P, NXC, XCH).transpose(2, 1, 0, 3)
        ).astype(f8)
        in_maps.append({"xt": xt_h, "w2": w2_h, "ccf": ccf, "ccb": ccb})

    res = run_bass_kernel_spmd(nc, in_maps, list(range(NCORES)))
    last_run = (nc, in_maps)
    # y[p, t] = out[128t + p]  ->  out = y.T.flatten()
    out = np.stack(
        [res.results[c]["y"].T.reshape(-1) for c in range(NCORES)], axis=0
    )
    return out.astype(np.float32)
